# revision 21
# baseline (speedup 1.0000x reference)
"""NestedMLP MoE-routed kernel for 8 TRN2 NeuronCores.

Strategy:
  - Host routes tokens by expert (argsort of expert_mask), splits each
    expert's tokens across the 8 cores (data-parallel), pads each
    per-core expert group to a common capacity so all cores run one SPMD
    program.
  - Activations are kept feature-major ("transposed", [feature, token])
    so both matmuls are natural lhsT.T @ rhs with the contraction dim on
    partitions, and the per-feature biases are per-partition (fusable
    into the ACT/DVE PSUM eviction).
  - Weights/activations are bf16 (f32 PSUM accumulation); biases are f32;
    the output is staged/stored bf16 and upcast to f32 on the host.
  - Per expert e (shift = 3-e): d_in = 1024>>shift, d_hid = 4*d_in,
    d_out = 1024>>shift, using the nested weight slices
    w1[:d_hid,:d_in], w2[:d_out,:d_hid].

DMA schedule (the kernel head is gated by HBM ring bandwidth ~350GB/s
AND by dma_start issue serialization ~0.65us each on the sync queue):
  - Inputs are packed into a handful of contiguous per-expert DRAM
    blocks so each is ONE dma_start with 128 fat descriptors:
      hdr    = [x-e0 | w1-e0 | w2-e0]   (everything expert 0 touches)
      bias   = [b1t | b2t]
      blkA_e = [w1-delta_e | x_e]       (gates expert e's matmul1)
      blkB_e = [w2-delta_e]             (gates expert e's matmul2)
    in strict need order hdr, bias, e1A, e1B, e2A, e2B, e3A, e3B.
  - Two tiny dummy loads sit between bias and e1A: the PE's single
    coalesced DMA wait empirically sweeps ~4 dma_starts past its actual
    dependency, and the dummies keep that sweep cheap.
  - All dma_starts ride the sync (SP) queue. A second issue queue
    (scalar) confuses Tile's coalesced DMA-completion waits and made the
    first matmul wait on the whole weight stream - do not split queues.
"""

import math
import sys
import types

sys.path.insert(0, "/opt/trn_rl_repo")

import ml_dtypes
import numpy as np

P = 128
E = 4
D = 1024
H = 4096
OUT = 1024
NCORES = 8
MLP_RATIO = 4

BF16 = ml_dtypes.bfloat16

# (d_in, d_hid, d_out) per expert
DIMS = [((D >> (E - 1 - e)), (D >> (E - 1 - e)) * MLP_RATIO, (OUT >> (E - 1 - e))) for e in range(E)]
CHUNK_W = 512  # token columns per matmul pass; 512 = one PSUM bank of f32
# dependency-free dummy matmuls: N_WARMUP before any real work (bridges
# engine-start to first-input-landing and warms the HAM clock gate),
# N_FILLER between expert 0 and expert 1 (bridges the ring-BW wait for
# expert 1's inputs without letting the PE idle long enough for the HAM
# to re-throttle).
N_WARMUP = 27
N_FILLER = 8
N_PADS = 3


def _round_up(v, m):
    return ((v + m - 1) // m) * m


def _tile_fmajor(a2d):
    """[F, C] -> [128, F//128, C] with row f = po*128 + pi."""
    f, c = a2d.shape
    return np.ascontiguousarray(a2d.reshape(f // P, P, c).transpose(1, 0, 2))


def _w_groups(nk_of, ncols_of, e):
    """(k0, k1, lo, hi) weight-group extents added by expert e on top of
    expert e-1's nested footprint."""
    nk_prev = nk_of(e - 1) if e > 0 else 0
    cols_prev = ncols_of(e - 1) if e > 0 else 0
    nk, cols = nk_of(e), ncols_of(e)
    groups = []
    if nk_prev and cols > cols_prev:
        groups.append((0, nk_prev, cols_prev, cols))
    if nk > nk_prev:
        groups.append((nk_prev, nk, 0, cols))
    return groups


def _nk1(e):
    return DIMS[e][0] // P


def _nm1(e):
    return DIMS[e][1] // P


def _nk2(e):
    return DIMS[e][1] // P


def _nm2(e):
    return DIMS[e][2] // P


def _wgroups_of(w, e):
    if w == "w1":
        return _w_groups(_nk1, lambda i: DIMS[i][1], e)
    return _w_groups(_nk2, lambda i: DIMS[i][2], e)


def _blk_layout(e, caps):
    """Column layout of blkA_e ([w1 groups | x]) and blkB_e ([w2 groups]):
    list of (kind, k0, k1, lo, hi, col_off) and total cols, per block."""
    a, off = [], 0
    for k0, k1, lo, hi in _wgroups_of("w1", e):
        a.append(("w1", k0, k1, lo, hi, off))
        off += (k1 - k0) * (hi - lo)
    a.append(("x", 0, _nk1(e), 0, caps[e], off))
    off += _nk1(e) * caps[e]
    a_cols = off
    b, off = [], 0
    for k0, k1, lo, hi in _wgroups_of("w2", e):
        b.append(("w2", k0, k1, lo, hi, off))
        off += (k1 - k0) * (hi - lo)
    return a, a_cols, b, off


def _chunk_plan(e, caps):
    plan, c0 = [], 0
    while c0 < caps[e]:
        cn = min(CHUNK_W, caps[e] - c0)
        plan.append((c0, cn))
        c0 += cn
    return plan


def _build_graph(caps):
    """Build the SPMD Bass graph for per-core per-expert capacities `caps`."""
    import concourse.mybir as mybir
    import concourse.tile as tile
    from concourse import bacc

    f32 = mybir.dt.float32
    bf16 = mybir.dt.bfloat16
    Gelu = mybir.ActivationFunctionType.Gelu

    offs = np.concatenate([[0], np.cumsum(caps)]).astype(int)
    ctot = int(offs[-1])
    hdr_x, hdr_w1 = caps[0], DIMS[0][1]
    # [x-e0 | w1-e0 | w2-e0 | b1t,b2t byte-punned to bf16 cols]
    boff = caps[0] + DIMS[0][1] + (DIMS[0][1] // P) * DIMS[0][2]
    hdr_cols = boff + 2 * (H // P + OUT // P)

    nc = bacc.Bacc(None, target_bir_lowering=False, debug=False)
    hdr_d = nc.declare_dram_parameter("hdr", [P, hdr_cols], bf16, isOutput=False)
    pad_d = [
        nc.declare_dram_parameter(f"pad{i}", [P, 8], bf16, isOutput=False) for i in range(N_PADS)
    ]
    blk_d = {}
    for e in range(1, E):
        _, a_cols, _, b_cols = _blk_layout(e, caps)
        blk_d[(e, "A")] = nc.declare_dram_parameter(f"blkA{e}", [P, a_cols], bf16, isOutput=False)
        blk_d[(e, "B")] = nc.declare_dram_parameter(f"blkB{e}", [P, b_cols], bf16, isOutput=False)
    y_d = nc.declare_dram_parameter("yt", [P, OUT // P, ctot], bf16, isOutput=True)

    with tile.TileContext(nc) as tc:
        with (
            tc.tile_pool(name="wpool", bufs=1) as wpool,
            tc.tile_pool(name="hpool", bufs=1) as hpool,
            tc.tile_pool(name="ypool", bufs=2) as ypool,
            tc.tile_pool(name="pspool", bufs=8, space="PSUM") as pspool,
        ):
            wu = wpool.tile([P, P], bf16, tag="warmup")
            nc.vector.memset(wu[:], 0.0)
            wact = wpool.tile([P, P], bf16, tag="warmact")
            # dummy activation: loads the ACT Gelu table before the first
            # real gelu needs it (table load is ~1.3us)
            nc.scalar.activation(wact[:], wu[:], Gelu, bias=0.0)

            def warm_mms(n):
                for _ in range(n):
                    wps = pspool.tile([P, P], f32, tag="ps")
                    nc.tensor.matmul(wps[:], wu[:], wu[:], start=True, stop=True)

            warm_mms(N_WARMUP)

            # ---- input DMA issue, strict need order, all on sync ----
            hdr = wpool.tile([P, hdr_cols], bf16, tag="hdr")
            nc.sync.dma_start(hdr[:], hdr_d[:])
            pads = []
            for i in range(N_PADS):
                pt = wpool.tile([P, 8], bf16, tag=f"pad{i}")
                nc.sync.dma_start(pt[:], pad_d[i][:])
                pads.append(pt)

            def b1s(m):
                return hdr[:, boff + 2 * m : boff + 2 * m + 2].bitcast(f32)

            def b2s(m2):
                o = boff + 2 * (H // P)
                return hdr[:, o + 2 * m2 : o + 2 * m2 + 2].bitcast(f32)

            # w1x/w2x: k-tile index -> [(lo, hi, 2d slicer fn(k, a, b))]
            w1x = {0: [(0, DIMS[0][1], lambda k, a, b: hdr[:, hdr_x + a : hdr_x + b])]}
            w2x = {
                k: [
                    (
                        0,
                        DIMS[0][2],
                        lambda k2, a, b, _k=k: hdr[
                            :,
                            hdr_x + hdr_w1 + _k * DIMS[0][2] + a : hdr_x
                            + hdr_w1
                            + _k * DIMS[0][2]
                            + b,
                        ],
                    )
                ]
                for k in range(DIMS[0][1] // P)
            }
            xs = {}  # e -> fn(k, a, b)

            def _load_blk(e, which):
                a, a_cols, b, b_cols = _blk_layout(e, caps)
                ents, cols = (a, a_cols) if which == "A" else (b, b_cols)
                t = wpool.tile([P, cols], bf16, tag=f"blk{which}{e}")
                nc.sync.dma_start(t[:], blk_d[(e, which)][:])
                for kind, k0, k1, lo, hi, off in ents:
                    w = hi - lo

                    def fn(k, aa, bb, _t=t, _off=off, _k0=k0, _w=w):
                        base = _off + (k - _k0) * _w
                        return _t[:, base + aa : base + bb]

                    if kind == "x":
                        xs[e] = fn
                    else:
                        xd = w1x if kind == "w1" else w2x
                        for k in range(k0, k1):
                            xd.setdefault(k, []).append((lo, hi, fn))

            for e in range(1, E):
                _load_blk(e, "A")
                _load_blk(e, "B")

            def wslice(xdict, k, m):
                """[128, 128] lhsT slice for feature cols [m*128,(m+1)*128)."""
                lo_c, hi_c = m * P, (m + 1) * P
                for lo, hi, fn in xdict[k]:
                    if lo <= lo_c and hi_c <= hi:
                        return fn(k, lo_c - lo, hi_c - lo)
                raise AssertionError("weight slice not found")

            def xslice(e, c0, cn, k):
                if e == 0:
                    return hdr[:, c0 : c0 + cn]
                return xs[e](k, c0, c0 + cn)

            # ---- compute ----
            chunks = [(e, c0, cn) for e in range(E) for c0, cn in _chunk_plan(e, caps)]
            last = chunks[-1]
            for e, c0, cn in chunks:
                nk1, nm1 = _nk1(e), _nm1(e)
                nk2, nm2 = _nk2(e), _nm2(e)
                col = offs[e] + c0
                ht = hpool.tile([P, nm1, cn], bf16, tag="ht")
                for m in range(nm1):
                    ps = pspool.tile([P, cn], f32, tag="ps")
                    for k in range(nk1):
                        nc.tensor.matmul(
                            ps[:],
                            wslice(w1x, k, m),
                            xslice(e, c0, cn, k),
                            start=(k == 0),
                            stop=(k == nk1 - 1),
                        )
                    nc.scalar.activation(ht[:, m, :], ps[:], Gelu, bias=b1s(m))
                if e == 0 and c0 == 0:
                    # PE busy-filler while expert 1's inputs stream in
                    warm_mms(N_FILLER)
                if e == 1 and c0 == 0:
                    # consume the pads (so the loads aren't elided as dead)
                    # well after their data has landed - zero stall here
                    for pt in pads:
                        pps = pspool.tile([P, 8], f32, tag="ps")
                        nc.tensor.matmul(pps[:], wu[:], pt[:], start=True, stop=True)
                for m2 in range(nm2):
                    # the very last accumulation group is split so the bulk's
                    # bias-add + store overlap the final quarter's matmuls
                    # (shorter kernel tail after the final matmul)
                    final = (e, c0, cn) == last and m2 == nm2 - 1
                    csplits = (
                        ((0, 3 * cn // 4), (3 * cn // 4, cn)) if final and cn >= 256 else ((0, cn),)
                    )
                    for s0, s1 in csplits:
                        ps = pspool.tile([P, s1 - s0], f32, tag="ps")
                        for k2 in range(nk2):
                            nc.tensor.matmul(
                                ps[:],
                                wslice(w2x, k2, m2),
                                ht[:, k2, s0:s1],
                                start=(k2 == 0),
                                stop=(k2 == nk2 - 1),
                            )
                        # bias-add evicts PSUM to a bf16 SBUF slab (half the
                        # staging memory + output DMA bytes of f32); each
                        # slab streams out as soon as it's ready
                        yt = ypool.tile([P, s1 - s0], bf16, tag=f"yt{s0 if final else ''}")
                        nc.vector.tensor_scalar_add(yt[:], ps[:], b2s(m2))
                        nc.sync.dma_start(y_d[:, m2, col + s0 : col + s1], yt[:])

    nc.compile()
    return nc, ctot, offs


def _ensure_ntff_hook_importable():
    """bass_utils' trace path imports antenv.axon_hooks, which some images
    lack; install a working shim so tracing (e.g. BASS_TRACE=1 in the
    environment) degrades gracefully instead of crashing. No-op when the
    real module exists."""
    try:
        import antenv.axon_hooks  # noqa: F401
        return
    except ImportError:
        pass
    holder = {"hook": None}
    m = types.ModuleType("antenv.axon_hooks")
    m.set_axon_ntff_profile_hook = lambda h: holder.__setitem__("hook", h)
    m.get_axon_ntff_profile_hook = lambda: holder["hook"]
    sys.modules["antenv.axon_hooks"] = m
    try:
        from trn_agent_boot.trn_boot import _ntff_profile_via_ctypes

        m.set_axon_ntff_profile_hook(_ntff_profile_via_ctypes("/opt/axon/libaxon_pjrt.so"))
    except Exception:
        pass  # hook stays None; bass_utils logs and skips tracing


def kernel(x, expert_mask, w1, b1, w2, b2):
    _ensure_ntff_hook_importable()
    from concourse.bass_utils import run_bass_kernel_spmd

    B, N, _ = x.shape
    T = B * N
    xf = np.asarray(x, dtype=np.float32).reshape(T, D)
    mask = np.asarray(expert_mask).reshape(T).astype(np.int64)

    # --- host routing ---
    ids_by_e = [np.nonzero(mask == e)[0] for e in range(E)]
    counts = [len(i) for i in ids_by_e]
    caps = [max(64, _round_up(math.ceil(c / NCORES), 64)) for c in counts]
    core_ids = [[None] * E for _ in range(NCORES)]
    for e in range(E):
        parts = np.array_split(ids_by_e[e], NCORES)
        for c in range(NCORES):
            assert len(parts[c]) <= caps[e]
            core_ids[c][e] = parts[c]

    nc, ctot, offs = _build_graph(caps)

    # --- host input prep ---
    w1t = _tile_fmajor(np.asarray(w1, np.float32).T).astype(BF16)  # [128, 8, H]
    w2t = _tile_fmajor(np.asarray(w2, np.float32).T).astype(BF16)  # [128, 32, OUT]
    b1t = np.ascontiguousarray(np.asarray(b1, np.float32).reshape(H // P, P).T)
    b2t = np.ascontiguousarray(np.asarray(b2, np.float32).reshape(OUT // P, P).T)

    # header: w1-e0 [128, 512], w2-e0 k-planes [128, 4*128], then the f32
    # biases byte-punned into bf16 columns (device bitcasts them back)
    hdr_w = np.concatenate(
        [w1t[:, 0, : DIMS[0][1]]]
        + [w2t[:, k, : DIMS[0][2]] for k in range(DIMS[0][1] // P)]
        + [b1t.view(BF16), b2t.view(BF16)],
        axis=1,
    )
    pad = np.zeros((P, 8), BF16)

    in_maps = []
    for c in range(NCORES):
        xg = np.zeros((ctot, D), np.float32)
        for e in range(E):
            ids = core_ids[c][e]
            xg[offs[e] : offs[e] + len(ids)] = xf[ids]
        xt = _tile_fmajor(xg.T).astype(BF16)  # [128, 8, ctot]
        m = {f"pad{i}": pad for i in range(N_PADS)}
        m["hdr"] = np.ascontiguousarray(
            np.concatenate([xt[:, 0, offs[0] : offs[1]], hdr_w], axis=1)
        )
        for e in range(1, E):
            a, a_cols, b, b_cols = _blk_layout(e, caps)
            for which, ents, cols in (("A", a, a_cols), ("B", b, b_cols)):
                parts_ = []
                for kind, k0, k1, lo, hi, off in ents:
                    if kind == "x":
                        parts_.append(xt[:, : _nk1(e), offs[e] : offs[e] + caps[e]].reshape(P, -1))
                    else:
                        src = w1t if kind == "w1" else w2t
                        parts_.append(np.ascontiguousarray(src[:, k0:k1, lo:hi]).reshape(P, -1))
                m[f"blk{which}{e}"] = np.ascontiguousarray(np.concatenate(parts_, axis=1))
        in_maps.append(m)

    res = run_bass_kernel_spmd(nc, in_maps, list(range(NCORES)))

    # --- host output assembly ---
    y = np.zeros((T, OUT), np.float32)
    for c in range(NCORES):
        yr = np.asarray(res.results[c]["yt"]).astype(np.float32)  # [128, 8, ctot]
        yfull = yr.transpose(1, 0, 2).reshape(OUT, ctot)
        for e in range(E):
            d_out = DIMS[e][2]
            ids = core_ids[c][e]
            if len(ids):
                y[ids, :d_out] = yfull[:d_out, offs[e] : offs[e] + len(ids)].T
    return y.reshape(B, N, OUT)


# revision 22
# speedup vs baseline: 1.1882x; 1.1882x over previous
"""NestedMLP MoE-routed kernel for 8 TRN2 NeuronCores.

Strategy:
  - Host routes tokens by expert (argsort of expert_mask), splits each
    expert's tokens across the 8 cores (data-parallel), pads each
    per-core expert group to a common capacity so all cores run one SPMD
    program.
  - Activations are kept feature-major ("transposed", [feature, token])
    so both matmuls are natural lhsT.T @ rhs with the contraction dim on
    partitions, and the per-feature biases are per-partition (fusable
    into the ACT/DVE PSUM eviction).
  - Weights/activations are bf16 (f32 PSUM accumulation); biases are f32;
    the output is staged/stored bf16 and upcast to f32 on the host.
  - Per expert e (shift = 3-e): d_in = 1024>>shift, d_hid = 4*d_in,
    d_out = 1024>>shift, using the nested weight slices
    w1[:d_hid,:d_in], w2[:d_out,:d_hid].

DMA schedule (the kernel is HBM-read-bound at the head: ~350 GB/s from
~8.4us after launch, 20.5 MB of inputs):
  - Every input tensor a dma_start touches is staged in DRAM as its own
    contiguous [128, ...] block, so one dma_start = 128 fat descriptors.
    (Strided slices of a big tensor cost 128*nk thin descriptors; their
    generation on the issue queue, ~5ns/line, was a head bottleneck.)
  - Expert 0's entire input set (x + w1 + w2 slices) is one "header"
    dma_start so the PE can start real work as soon as one transfer
    lands.
  - All dma_starts ride the sync (SP) queue in strict need order; x
    chunks for the big experts are emitted AFTER the weight groups of
    the preceding experts (w1-e3 before x-e3) because the weight bytes
    gate the compute earlier than the x bytes do.  A second issue queue
    (scalar) confuses Tile's coalesced DMA-completion waits and made the
    first matmul wait on the whole weight stream - do not split queues.
"""

import math
import sys
import types

sys.path.insert(0, "/opt/trn_rl_repo")

import ml_dtypes
import numpy as np

P = 128
E = 4
D = 1024
H = 4096
OUT = 1024
NCORES = 8
MLP_RATIO = 4

BF16 = ml_dtypes.bfloat16

# (d_in, d_hid, d_out) per expert
DIMS = [((D >> (E - 1 - e)), (D >> (E - 1 - e)) * MLP_RATIO, (OUT >> (E - 1 - e))) for e in range(E)]
CHUNK_W = 512  # token columns per matmul pass; 512 = one PSUM bank of f32
# dependency-free dummy matmuls: N_WARMUP before any real work (bridges
# engine-start to first-input-landing and warms the HAM clock gate),
# N_FILLER between expert 0 and expert 1 (bridges the ring-BW wait for
# expert 1's weights without letting the PE idle long enough for the HAM
# to re-throttle).
N_WARMUP = 20
N_FILLER = 12


def _round_up(v, m):
    return ((v + m - 1) // m) * m


def _tile_fmajor(a2d):
    """[F, C] -> [128, F//128, C] with row f = po*128 + pi."""
    f, c = a2d.shape
    return np.ascontiguousarray(a2d.reshape(f // P, P, c).transpose(1, 0, 2))


def _w_groups(nk_of, ncols_of, e, split_halves):
    """(k0, k1, lo, hi) weight-group extents added by expert e on top of
    expert e-1's nested footprint."""
    nk_prev = nk_of(e - 1) if e > 0 else 0
    cols_prev = ncols_of(e - 1) if e > 0 else 0
    nk, cols = nk_of(e), ncols_of(e)
    groups = []
    if nk_prev and cols > cols_prev:
        groups.append((0, nk_prev, cols_prev, cols))
    if nk > nk_prev:
        halves = 2 if split_halves else 1
        step = cols // halves
        for plo in range(0, cols, step):
            groups.append((nk_prev, nk, plo, plo + step))
    return groups


def _nk1(e):
    return DIMS[e][0] // P


def _nm1(e):
    return DIMS[e][1] // P


def _nk2(e):
    return DIMS[e][1] // P


def _nm2(e):
    return DIMS[e][2] // P


def _wgroups_of(w, e):
    if w == "w1":
        return _w_groups(_nk1, lambda i: DIMS[i][1], e, e in (1, 2))
    return _w_groups(_nk2, lambda i: DIMS[i][2], e, e in (1, 2))


def _chunk_plan(e, caps):
    plan, c0 = [], 0
    while c0 < caps[e]:
        cn = min(CHUNK_W, caps[e] - c0)
        plan.append((c0, cn))
        c0 += cn
    return plan


# header layout (expert 0's whole input set, one contiguous DRAM block):
# [x-e0 (caps[0] cols) | w1-e0 (512 cols) | w2-e0 (4 k-planes x 128 cols)]
def _build_graph(caps):
    """Build the SPMD Bass graph for per-core per-expert capacities `caps`."""
    import concourse.mybir as mybir
    import concourse.tile as tile
    from concourse import bacc

    f32 = mybir.dt.float32
    bf16 = mybir.dt.bfloat16
    Gelu = mybir.ActivationFunctionType.Gelu

    offs = np.concatenate([[0], np.cumsum(caps)]).astype(int)
    ctot = int(offs[-1])
    hdr_x, hdr_w1 = caps[0], DIMS[0][1]
    hdr_cols = caps[0] + DIMS[0][1] + (DIMS[0][1] // P) * DIMS[0][2]

    nc = bacc.Bacc(None, target_bir_lowering=False, debug=False)
    hdr_d = nc.declare_dram_parameter("hdr", [P, hdr_cols], bf16, isOutput=False)
    b1_d = nc.declare_dram_parameter("b1t", [P, H // P], f32, isOutput=False)
    b2_d = nc.declare_dram_parameter("b2t", [P, OUT // P], f32, isOutput=False)
    x_d = {
        e: nc.declare_dram_parameter(f"x_{e}", [P, _nk1(e), caps[e]], bf16, isOutput=False)
        for e in range(1, E)
    }
    wg_d = {
        (w, e, k0, lo): nc.declare_dram_parameter(
            f"{w}g_{e}_{k0}_{lo}", [P, k1 - k0, hi - lo], bf16, isOutput=False
        )
        for e in range(1, E)
        for w in ("w1", "w2")
        for k0, k1, lo, hi in _wgroups_of(w, e)
    }
    y_d = nc.declare_dram_parameter("yt", [P, OUT // P, ctot], bf16, isOutput=True)

    with tile.TileContext(nc) as tc:
        with (
            tc.tile_pool(name="wpool", bufs=1) as wpool,
            tc.tile_pool(name="xpool", bufs=1) as xpool,
            tc.tile_pool(name="hpool", bufs=1) as hpool,
            tc.tile_pool(name="ypool", bufs=2) as ypool,
            tc.tile_pool(name="pspool", bufs=8, space="PSUM") as pspool,
        ):
            wu = wpool.tile([P, P], bf16, tag="warmup")
            nc.vector.memset(wu[:], 0.0)
            wact = wpool.tile([P, P], bf16, tag="warmact")
            # dummy activation: loads the ACT Gelu table before the first
            # real gelu needs it (table load is ~1.3us)
            nc.scalar.activation(wact[:], wu[:], Gelu, bias=0.0)

            def warm_mms(n):
                for _ in range(n):
                    wps = pspool.tile([P, P], f32, tag="ps")
                    nc.tensor.matmul(wps[:], wu[:], wu[:], start=True, stop=True)

            warm_mms(N_WARMUP)

            b1sb = wpool.tile([P, H // P], f32, tag="b1")
            b2sb = wpool.tile([P, OUT // P], f32, tag="b2")

            # ---- input DMA issue, strict need order, all on sync ----
            hdr = wpool.tile([P, hdr_cols], bf16, tag="hdr")
            nc.sync.dma_start(hdr[:], hdr_d[:])
            nc.sync.dma_start(b1sb[:], b1_d[:])
            nc.sync.dma_start(b2sb[:], b2_d[:])

            # w1x/w2x: k-tile index -> [(lo, hi, k0, 2d-slicer)]
            w1x = {0: [(0, DIMS[0][1], 0, lambda k, a, b: hdr[:, hdr_x + a : hdr_x + b])]}
            w2x = {
                k: [
                    (
                        0,
                        DIMS[0][2],
                        0,
                        lambda k2, a, b, _k=k: hdr[
                            :,
                            hdr_x + hdr_w1 + _k * DIMS[0][2] + a : hdr_x + hdr_w1 + _k * DIMS[0][2] + b,
                        ],
                    )
                ]
                for k in range(DIMS[0][1] // P)
            }
            xts = {0: hdr}

            def _load_wgroups(w, e, xdict):
                for k0, k1, lo, hi in _wgroups_of(w, e):
                    t = wpool.tile([P, k1 - k0, hi - lo], bf16, tag=f"{w}_{k0}_{lo}")
                    nc.sync.dma_start(t[:], wg_d[(w, e, k0, lo)][:])
                    for k in range(k0, k1):
                        xdict.setdefault(k, []).append(
                            (lo, hi, k0, lambda k2, a, b, _t=t, _k0=k0: _t[:, k2 - _k0, a:b])
                        )

            def _load_x(e):
                xt = xpool.tile([P, _nk1(e), caps[e]], bf16, tag=f"xt_{e}")
                nc.sync.dma_start(xt[:], x_d[e][:])
                xts[e] = xt

            # weight groups ahead of the same expert's x: the weight bytes
            # gate the matmul phases earlier than the x bytes do, and the
            # PE's coalesced first wait sweeps in a few dma_starts past the
            # header - keep those small.
            _load_wgroups("w1", 1, w1x)
            _load_x(1)
            _load_wgroups("w2", 1, w2x)
            _load_wgroups("w1", 2, w1x)
            _load_x(2)
            _load_wgroups("w2", 2, w2x)
            _load_wgroups("w1", 3, w1x)
            _load_x(3)
            _load_wgroups("w2", 3, w2x)

            def wslice(xdict, k, m):
                """[128, 128] lhsT slice for feature cols [m*128,(m+1)*128)."""
                lo_c, hi_c = m * P, (m + 1) * P
                for lo, hi, k0, fn in xdict[k]:
                    if lo <= lo_c and hi_c <= hi:
                        return fn(k, lo_c - lo, hi_c - lo)
                raise AssertionError("weight slice not found")

            def xslice(e, c0, cn, k):
                if e == 0:
                    return hdr[:, c0:c0+cn]
                return xts[e][:, k, c0 : c0 + cn]

            # ---- compute ----
            chunks = [(e, c0, cn) for e in range(E) for c0, cn in _chunk_plan(e, caps)]
            last = chunks[-1]
            for e, c0, cn in chunks:
                nk1, nm1 = _nk1(e), _nm1(e)
                nk2, nm2 = _nk2(e), _nm2(e)
                col = offs[e] + c0
                ht = hpool.tile([P, nm1, cn], bf16, tag="ht")
                for m in range(nm1):
                    ps = pspool.tile([P, cn], f32, tag="ps")
                    for k in range(nk1):
                        nc.tensor.matmul(
                            ps[:],
                            wslice(w1x, k, m),
                            xslice(e, c0, cn, k),
                            start=(k == 0),
                            stop=(k == nk1 - 1),
                        )
                    nc.scalar.activation(ht[:, m, :], ps[:], Gelu, bias=b1sb[:, m : m + 1])
                if e == 0 and c0 == 0:
                    # PE busy-filler while expert 1's weights stream in
                    warm_mms(N_FILLER)
                for m2 in range(nm2):
                    # the very last accumulation group is split into two
                    # column halves so the first half's bias-add + store
                    # overlap the second half's matmuls (shorter kernel
                    # tail after the final matmul)
                    final = (e, c0, cn) == last and m2 == nm2 - 1
                    csplits = ((0, 3 * cn // 4), (3 * cn // 4, cn)) if final and cn >= 256 else ((0, cn),)
                    for s0, s1 in csplits:
                        ps = pspool.tile([P, s1 - s0], f32, tag="ps")
                        for k2 in range(nk2):
                            nc.tensor.matmul(
                                ps[:],
                                wslice(w2x, k2, m2),
                                ht[:, k2, s0:s1],
                                start=(k2 == 0),
                                stop=(k2 == nk2 - 1),
                            )
                        # bias-add evicts PSUM to a bf16 SBUF slab (half the
                        # staging memory + output DMA bytes of f32); each
                        # slab streams out as soon as it's ready
                        yt = ypool.tile([P, s1 - s0], bf16, tag=f"yt{s0 if final else ''}")
                        nc.vector.tensor_scalar_add(yt[:], ps[:], b2sb[:, m2 : m2 + 1])
                        nc.sync.dma_start(y_d[:, m2, col + s0 : col + s1], yt[:])

    nc.compile()
    return nc, ctot, offs


def _ensure_ntff_hook_importable():
    """bass_utils' trace path imports antenv.axon_hooks, which some images
    lack; install a working shim so tracing (e.g. BASS_TRACE=1 in the
    environment) degrades gracefully instead of crashing. No-op when the
    real module exists."""
    try:
        import antenv.axon_hooks  # noqa: F401
        return
    except ImportError:
        pass
    holder = {"hook": None}
    m = types.ModuleType("antenv.axon_hooks")
    m.set_axon_ntff_profile_hook = lambda h: holder.__setitem__("hook", h)
    m.get_axon_ntff_profile_hook = lambda: holder["hook"]
    sys.modules["antenv.axon_hooks"] = m
    try:
        from trn_agent_boot.trn_boot import _ntff_profile_via_ctypes

        m.set_axon_ntff_profile_hook(_ntff_profile_via_ctypes("/opt/axon/libaxon_pjrt.so"))
    except Exception:
        pass  # hook stays None; bass_utils logs and skips tracing


def kernel(x, expert_mask, w1, b1, w2, b2):
    _ensure_ntff_hook_importable()
    from concourse.bass_utils import run_bass_kernel_spmd

    B, N, _ = x.shape
    T = B * N
    xf = np.asarray(x, dtype=np.float32).reshape(T, D)
    mask = np.asarray(expert_mask).reshape(T).astype(np.int64)

    # --- host routing ---
    ids_by_e = [np.nonzero(mask == e)[0] for e in range(E)]
    counts = [len(i) for i in ids_by_e]
    caps = [max(64, _round_up(math.ceil(c / NCORES), 64)) for c in counts]
    core_ids = [[None] * E for _ in range(NCORES)]
    for e in range(E):
        parts = np.array_split(ids_by_e[e], NCORES)
        for c in range(NCORES):
            assert len(parts[c]) <= caps[e]
            core_ids[c][e] = parts[c]

    nc, ctot, offs = _build_graph(caps)

    # --- host input prep ---
    w1t = _tile_fmajor(np.asarray(w1, np.float32).T).astype(BF16)  # [128, 8, H]
    w2t = _tile_fmajor(np.asarray(w2, np.float32).T).astype(BF16)  # [128, 32, OUT]
    b1t = np.ascontiguousarray(np.asarray(b1, np.float32).reshape(H // P, P).T)
    b2t = np.ascontiguousarray(np.asarray(b2, np.float32).reshape(OUT // P, P).T)

    wmap = {}
    for e in range(1, E):
        for w in ("w1", "w2"):
            src = w1t if w == "w1" else w2t
            for k0, k1, lo, hi in _wgroups_of(w, e):
                wmap[f"{w}g_{e}_{k0}_{lo}"] = np.ascontiguousarray(src[:, k0:k1, lo:hi])

    # header weight part: w1-e0 [128, 512] then w2-e0 k-planes [128, 4*128]
    hdr_w = np.concatenate(
        [w1t[:, 0, : DIMS[0][1]]]
        + [w2t[:, k, : DIMS[0][2]] for k in range(DIMS[0][1] // P)],
        axis=1,
    )

    in_maps = []
    for c in range(NCORES):
        xg = np.zeros((ctot, D), np.float32)
        for e in range(E):
            ids = core_ids[c][e]
            xg[offs[e] : offs[e] + len(ids)] = xf[ids]
        xt = _tile_fmajor(xg.T).astype(BF16)  # [128, 8, ctot]
        m = {"b1t": b1t, "b2t": b2t, **wmap}
        m["hdr"] = np.ascontiguousarray(
            np.concatenate([xt[:, 0, offs[0] : offs[1]], hdr_w], axis=1)
        )
        for e in range(1, E):
            m[f"x_{e}"] = np.ascontiguousarray(
                xt[:, : _nk1(e), offs[e] : offs[e] + caps[e]]
            )
        in_maps.append(m)

    res = run_bass_kernel_spmd(nc, in_maps, list(range(NCORES)))

    # --- host output assembly ---
    y = np.zeros((T, OUT), np.float32)
    for c in range(NCORES):
        yr = np.asarray(res.results[c]["yt"]).astype(np.float32)  # [128, 8, ctot]
        yfull = yr.transpose(1, 0, 2).reshape(OUT, ctot)
        for e in range(E):
            d_out = DIMS[e][2]
            ids = core_ids[c][e]
            if len(ids):
                y[ids, :d_out] = yfull[:d_out, offs[e] : offs[e] + len(ids)].T
    return y.reshape(B, N, OUT)


# revision 29
# speedup vs baseline: 1.1903x; 1.0018x over previous
"""NestedMLP MoE-routed kernel for 8 TRN2 NeuronCores.

Strategy:
  - Host routes tokens by expert (argsort of expert_mask), splits each
    expert's tokens across the 8 cores (data-parallel), pads each
    per-core expert group to a common capacity so all cores run one SPMD
    program.
  - Activations are kept feature-major ("transposed", [feature, token])
    so both matmuls are natural lhsT.T @ rhs with the contraction dim on
    partitions, and the per-feature biases are per-partition (fusable
    into the ACT/DVE PSUM eviction).
  - Weights/activations are bf16 (f32 PSUM accumulation); biases are f32;
    the output is staged/stored bf16 and upcast to f32 on the host.
  - Per expert e (shift = 3-e): d_in = 1024>>shift, d_hid = 4*d_in,
    d_out = 1024>>shift, using the nested weight slices
    w1[:d_hid,:d_in], w2[:d_out,:d_hid].

DMA schedule (the kernel is HBM-read-bound at the head: ~350 GB/s from
~8.4us after launch, 20.5 MB of inputs):
  - Every input tensor a dma_start touches is staged in DRAM as its own
    contiguous [128, ...] block, so one dma_start = 128 fat descriptors.
    (Strided slices of a big tensor cost 128*nk thin descriptors; their
    generation on the issue queue, ~5ns/line, was a head bottleneck.)
  - Expert 0's entire input set (x + w1 + w2 slices) is one "header"
    dma_start so the PE can start real work as soon as one transfer
    lands.
  - All dma_starts ride the sync (SP) queue in strict need order; x
    chunks for the big experts are emitted AFTER the weight groups of
    the preceding experts (w1-e3 before x-e3) because the weight bytes
    gate the compute earlier than the x bytes do.  A second issue queue
    (scalar) confuses Tile's coalesced DMA-completion waits and made the
    first matmul wait on the whole weight stream - do not split queues.
"""

import math
import sys
import types

sys.path.insert(0, "/opt/trn_rl_repo")

import ml_dtypes
import numpy as np

P = 128
E = 4
D = 1024
H = 4096
OUT = 1024
NCORES = 8
MLP_RATIO = 4

BF16 = ml_dtypes.bfloat16

# (d_in, d_hid, d_out) per expert
DIMS = [((D >> (E - 1 - e)), (D >> (E - 1 - e)) * MLP_RATIO, (OUT >> (E - 1 - e))) for e in range(E)]
CHUNK_W = 512  # token columns per matmul pass; 512 = one PSUM bank of f32
# dependency-free dummy matmuls: N_WARMUP before any real work (bridges
# engine-start to first-input-landing and warms the HAM clock gate),
# N_FILLER between expert 0 and expert 1 (bridges the ring-BW wait for
# expert 1's weights without letting the PE idle long enough for the HAM
# to re-throttle).
N_WARMUP = 30
N_FILLER = 12


def _round_up(v, m):
    return ((v + m - 1) // m) * m


def _tile_fmajor(a2d):
    """[F, C] -> [128, F//128, C] with row f = po*128 + pi."""
    f, c = a2d.shape
    return np.ascontiguousarray(a2d.reshape(f // P, P, c).transpose(1, 0, 2))


def _w_groups(nk_of, ncols_of, e, split_halves):
    """(k0, k1, lo, hi) weight-group extents added by expert e on top of
    expert e-1's nested footprint."""
    nk_prev = nk_of(e - 1) if e > 0 else 0
    cols_prev = ncols_of(e - 1) if e > 0 else 0
    nk, cols = nk_of(e), ncols_of(e)
    groups = []
    if nk_prev and cols > cols_prev:
        groups.append((0, nk_prev, cols_prev, cols))
    if nk > nk_prev:
        halves = 2 if split_halves else 1
        step = cols // halves
        for plo in range(0, cols, step):
            groups.append((nk_prev, nk, plo, plo + step))
    return groups


def _nk1(e):
    return DIMS[e][0] // P


def _nm1(e):
    return DIMS[e][1] // P


def _nk2(e):
    return DIMS[e][1] // P


def _nm2(e):
    return DIMS[e][2] // P


def _wgroups_of(w, e):
    if w == "w1":
        return _w_groups(_nk1, lambda i: DIMS[i][1], e, e in (1, 2))
    return _w_groups(_nk2, lambda i: DIMS[i][2], e, e in (1, 2))


def _chunk_plan(e, caps):
    plan, c0 = [], 0
    while c0 < caps[e]:
        cn = min(CHUNK_W, caps[e] - c0)
        plan.append((c0, cn))
        c0 += cn
    return plan


# header layout (expert 0's whole input set, one contiguous DRAM block):
# [x-e0 (caps[0] cols) | w1-e0 (512 cols) | w2-e0 (4 k-planes x 128 cols)]
def _build_graph(caps):
    """Build the SPMD Bass graph for per-core per-expert capacities `caps`."""
    import concourse.mybir as mybir
    import concourse.tile as tile
    from concourse import bacc

    f32 = mybir.dt.float32
    bf16 = mybir.dt.bfloat16
    Gelu = mybir.ActivationFunctionType.Gelu

    offs = np.concatenate([[0], np.cumsum(caps)]).astype(int)
    ctot = int(offs[-1])
    hdr_x, hdr_w1 = caps[0], DIMS[0][1]
    # [x-e0 | w1-e0 | w2-e0 | b1t,b2t byte-punned to bf16 cols]
    boff = caps[0] + DIMS[0][1] + (DIMS[0][1] // P) * DIMS[0][2]
    hdr_cols = boff + 2 * (H // P + OUT // P)

    nc = bacc.Bacc(None, target_bir_lowering=False, debug=False)
    hdr_d = nc.declare_dram_parameter("hdr", [P, hdr_cols], bf16, isOutput=False)
    x_d = {
        e: nc.declare_dram_parameter(f"x_{e}", [P, _nk1(e), caps[e]], bf16, isOutput=False)
        for e in range(1, E)
    }
    wg_d = {
        (w, e, k0, lo): nc.declare_dram_parameter(
            f"{w}g_{e}_{k0}_{lo}", [P, k1 - k0, hi - lo], bf16, isOutput=False
        )
        for e in range(1, E)
        for w in ("w1", "w2")
        for k0, k1, lo, hi in _wgroups_of(w, e)
    }
    y_d = nc.declare_dram_parameter("yt", [P, OUT // P, ctot], bf16, isOutput=True)

    with tile.TileContext(nc) as tc:
        with (
            tc.tile_pool(name="wpool", bufs=1) as wpool,
            tc.tile_pool(name="xpool", bufs=1) as xpool,
            tc.tile_pool(name="hpool", bufs=1) as hpool,
            tc.tile_pool(name="ypool", bufs=2) as ypool,
            tc.tile_pool(name="pspool", bufs=8, space="PSUM") as pspool,
        ):
            wu = wpool.tile([P, P], bf16, tag="warmup")
            nc.vector.memset(wu[:], 0.0)
            wact = wpool.tile([P, P], bf16, tag="warmact")
            # dummy activation: loads the ACT Gelu table before the first
            # real gelu needs it (table load is ~1.3us)
            nc.scalar.activation(wact[:], wu[:], Gelu, bias=0.0)

            def warm_mms(n):
                for _ in range(n):
                    wps = pspool.tile([P, P], f32, tag="ps")
                    nc.tensor.matmul(wps[:], wu[:], wu[:], start=True, stop=True)

            warm_mms(N_WARMUP)

            # ---- input DMA issue, strict need order, all on sync ----
            hdr = wpool.tile([P, hdr_cols], bf16, tag="hdr")
            nc.sync.dma_start(hdr[:], hdr_d[:])

            def b1s(m):
                return hdr[:, boff + 2 * m : boff + 2 * m + 2].bitcast(f32)

            def b2s(m2):
                o = boff + 2 * (H // P)
                return hdr[:, o + 2 * m2 : o + 2 * m2 + 2].bitcast(f32)

            # w1x/w2x: k-tile index -> [(lo, hi, k0, 2d-slicer)]
            w1x = {0: [(0, DIMS[0][1], 0, lambda k, a, b: hdr[:, hdr_x + a : hdr_x + b])]}
            w2x = {
                k: [
                    (
                        0,
                        DIMS[0][2],
                        0,
                        lambda k2, a, b, _k=k: hdr[
                            :,
                            hdr_x + hdr_w1 + _k * DIMS[0][2] + a : hdr_x + hdr_w1 + _k * DIMS[0][2] + b,
                        ],
                    )
                ]
                for k in range(DIMS[0][1] // P)
            }
            xts = {0: hdr}

            def _load_wgroups(w, e, xdict):
                for k0, k1, lo, hi in _wgroups_of(w, e):
                    t = wpool.tile([P, k1 - k0, hi - lo], bf16, tag=f"{w}_{k0}_{lo}")
                    nc.sync.dma_start(t[:], wg_d[(w, e, k0, lo)][:])
                    for k in range(k0, k1):
                        xdict.setdefault(k, []).append(
                            (lo, hi, k0, lambda k2, a, b, _t=t, _k0=k0: _t[:, k2 - _k0, a:b])
                        )

            def _load_x(e):
                xt = xpool.tile([P, _nk1(e), caps[e]], bf16, tag=f"xt_{e}")
                nc.sync.dma_start(xt[:], x_d[e][:])
                xts[e] = xt

            # weight groups ahead of the same expert's x: the weight bytes
            # gate the matmul phases earlier than the x bytes do, and the
            # PE's coalesced first wait sweeps in a few dma_starts past the
            # header - keep those small.
            _load_wgroups("w1", 1, w1x)
            _load_x(1)
            _load_wgroups("w2", 1, w2x)
            _load_wgroups("w1", 2, w1x)
            _load_x(2)
            _load_wgroups("w2", 2, w2x)
            _load_wgroups("w1", 3, w1x)
            _load_x(3)
            _load_wgroups("w2", 3, w2x)

            def wslice(xdict, k, m):
                """[128, 128] lhsT slice for feature cols [m*128,(m+1)*128)."""
                lo_c, hi_c = m * P, (m + 1) * P
                for lo, hi, k0, fn in xdict[k]:
                    if lo <= lo_c and hi_c <= hi:
                        return fn(k, lo_c - lo, hi_c - lo)
                raise AssertionError("weight slice not found")

            def xslice(e, c0, cn, k):
                if e == 0:
                    return hdr[:, c0:c0+cn]
                return xts[e][:, k, c0 : c0 + cn]

            # ---- compute ----
            chunks = [(e, c0, cn) for e in range(E) for c0, cn in _chunk_plan(e, caps)]
            last = chunks[-1]
            for e, c0, cn in chunks:
                nk1, nm1 = _nk1(e), _nm1(e)
                nk2, nm2 = _nk2(e), _nm2(e)
                col = offs[e] + c0
                ht = hpool.tile([P, nm1, cn], bf16, tag="ht")
                for m in range(nm1):
                    ps = pspool.tile([P, cn], f32, tag="ps")
                    for k in range(nk1):
                        nc.tensor.matmul(
                            ps[:],
                            wslice(w1x, k, m),
                            xslice(e, c0, cn, k),
                            start=(k == 0),
                            stop=(k == nk1 - 1),
                        )
                    nc.scalar.activation(ht[:, m, :], ps[:], Gelu, bias=b1s(m))
                if e == 0 and c0 == 0:
                    # PE busy-filler while expert 1's weights stream in
                    warm_mms(N_FILLER)
                for m2 in range(nm2):
                    # the very last accumulation group is split into two
                    # column halves so the first half's bias-add + store
                    # overlap the second half's matmuls (shorter kernel
                    # tail after the final matmul)
                    final = (e, c0, cn) == last and m2 == nm2 - 1
                    csplits = ((0, 3 * cn // 4), (3 * cn // 4, cn)) if final and cn >= 256 else ((0, cn),)
                    for s0, s1 in csplits:
                        ps = pspool.tile([P, s1 - s0], f32, tag="ps")
                        for k2 in range(nk2):
                            nc.tensor.matmul(
                                ps[:],
                                wslice(w2x, k2, m2),
                                ht[:, k2, s0:s1],
                                start=(k2 == 0),
                                stop=(k2 == nk2 - 1),
                            )
                        # bias-add evicts PSUM to a bf16 SBUF slab (half the
                        # staging memory + output DMA bytes of f32); each
                        # slab streams out as soon as it's ready
                        yt = ypool.tile([P, s1 - s0], bf16, tag=f"yt{s0 if final else ''}")
                        nc.vector.tensor_scalar_add(yt[:], ps[:], b2s(m2))
                        nc.sync.dma_start(y_d[:, m2, col + s0 : col + s1], yt[:])

    nc.compile()
    return nc, ctot, offs


def _ensure_ntff_hook_importable():
    """bass_utils' trace path imports antenv.axon_hooks, which some images
    lack; install a working shim so tracing (e.g. BASS_TRACE=1 in the
    environment) degrades gracefully instead of crashing. No-op when the
    real module exists."""
    try:
        import antenv.axon_hooks  # noqa: F401
        return
    except ImportError:
        pass
    holder = {"hook": None}
    m = types.ModuleType("antenv.axon_hooks")
    m.set_axon_ntff_profile_hook = lambda h: holder.__setitem__("hook", h)
    m.get_axon_ntff_profile_hook = lambda: holder["hook"]
    sys.modules["antenv.axon_hooks"] = m
    try:
        from trn_agent_boot.trn_boot import _ntff_profile_via_ctypes

        m.set_axon_ntff_profile_hook(_ntff_profile_via_ctypes("/opt/axon/libaxon_pjrt.so"))
    except Exception:
        pass  # hook stays None; bass_utils logs and skips tracing


def kernel(x, expert_mask, w1, b1, w2, b2):
    _ensure_ntff_hook_importable()
    from concourse.bass_utils import run_bass_kernel_spmd

    B, N, _ = x.shape
    T = B * N
    xf = np.asarray(x, dtype=np.float32).reshape(T, D)
    mask = np.asarray(expert_mask).reshape(T).astype(np.int64)

    # --- host routing ---
    ids_by_e = [np.nonzero(mask == e)[0] for e in range(E)]
    counts = [len(i) for i in ids_by_e]
    caps = [max(64, _round_up(math.ceil(c / NCORES), 64)) for c in counts]
    core_ids = [[None] * E for _ in range(NCORES)]
    for e in range(E):
        parts = np.array_split(ids_by_e[e], NCORES)
        for c in range(NCORES):
            assert len(parts[c]) <= caps[e]
            core_ids[c][e] = parts[c]

    nc, ctot, offs = _build_graph(caps)

    # --- host input prep ---
    w1t = _tile_fmajor(np.asarray(w1, np.float32).T).astype(BF16)  # [128, 8, H]
    w2t = _tile_fmajor(np.asarray(w2, np.float32).T).astype(BF16)  # [128, 32, OUT]
    b1t = np.ascontiguousarray(np.asarray(b1, np.float32).reshape(H // P, P).T)
    b2t = np.ascontiguousarray(np.asarray(b2, np.float32).reshape(OUT // P, P).T)

    wmap = {}
    for e in range(1, E):
        for w in ("w1", "w2"):
            src = w1t if w == "w1" else w2t
            for k0, k1, lo, hi in _wgroups_of(w, e):
                wmap[f"{w}g_{e}_{k0}_{lo}"] = np.ascontiguousarray(src[:, k0:k1, lo:hi])

    # header: w1-e0 [128, 512], w2-e0 k-planes [128, 4*128], then the f32
    # biases byte-punned into bf16 columns (device bitcasts them back)
    hdr_w = np.concatenate(
        [w1t[:, 0, : DIMS[0][1]]]
        + [w2t[:, k, : DIMS[0][2]] for k in range(DIMS[0][1] // P)]
        + [b1t.view(BF16), b2t.view(BF16)],
        axis=1,
    )

    in_maps = []
    for c in range(NCORES):
        xg = np.zeros((ctot, D), np.float32)
        for e in range(E):
            ids = core_ids[c][e]
            xg[offs[e] : offs[e] + len(ids)] = xf[ids]
        xt = _tile_fmajor(xg.T).astype(BF16)  # [128, 8, ctot]
        m = {**wmap}
        m["hdr"] = np.ascontiguousarray(
            np.concatenate([xt[:, 0, offs[0] : offs[1]], hdr_w], axis=1)
        )
        for e in range(1, E):
            m[f"x_{e}"] = np.ascontiguousarray(
                xt[:, : _nk1(e), offs[e] : offs[e] + caps[e]]
            )
        in_maps.append(m)

    res = run_bass_kernel_spmd(nc, in_maps, list(range(NCORES)))

    # --- host output assembly ---
    y = np.zeros((T, OUT), np.float32)
    for c in range(NCORES):
        yr = np.asarray(res.results[c]["yt"]).astype(np.float32)  # [128, 8, ctot]
        yfull = yr.transpose(1, 0, 2).reshape(OUT, ctot)
        for e in range(E):
            d_out = DIMS[e][2]
            ids = core_ids[c][e]
            if len(ids):
                y[ids, :d_out] = yfull[:d_out, offs[e] : offs[e] + len(ids)].T
    return y.reshape(B, N, OUT)
